# revision 36
# baseline (speedup 1.0000x reference)
"""Trainium2 Bass kernel for nn_Sage14FX (encoder-GRU-attention-choice-decoder).

Sharding: pure data parallel over batch B=32 across 8 NeuronCores (4 b/core).
All heavy matmuls run in fp32r (full PE rate). Host side does layout prep only
(im2col for conv1 input, weight reshapes, tiny weight-only constant folds).
"""
import contextlib
import os
import numpy as np

import concourse.bass as bass
import concourse.bacc as bacc
import concourse.tile as tile
import concourse.mybir as mybir
import concourse.bass_isa as bass_isa
from concourse.bass_utils import run_bass_kernel_spmd

F32 = mybir.dt.float32
F32R = mybir.dt.float32r
BF16 = mybir.dt.bfloat16
AF = mybir.ActivationFunctionType
ALU = mybir.AluOpType
AX = mybir.AxisListType

NCORES = 8
B, T, HI, WI, CIN = 32, 8, 32, 32, 11
HID, G, NCLS = 256, 20, 10
NB = int(os.environ.get("SAGE_NB", "4"))          # batches per core (dev: 1)
NIMG = NB * T
PX = HI * WI                                       # 1024
PAD = 34 * 34                                      # padded 32x32 image
P400 = G * G
DPAD = 22 * 22                                     # padded 20x20 image
K1 = 9 * CIN                                       # 99


def _build(dbg_names=()):
    nc = bacc.Bacc("TRN2", target_bir_lowering=False, debug=False,
                   num_devices=NCORES)

    def di(name, shape, dt=F32R):
        return nc.dram_tensor(name, list(shape), dt, kind="ExternalInput").ap()

    xim = di("xim", [NIMG, K1, PX])
    w1 = di("w1", [K1, 256])
    w2 = di("w2", [18, 128, 256], BF16)
    b1c = di("b1c", [128, 2], F32)
    b2c = di("b2c", [128, 2], F32)
    ggc = di("ggc", [128, 2], F32)
    lnbc = di("lnbc", [128, 2], F32)
    gruk = di("gruk", [2, 128, 768])
    grurk = di("grurk", [2, 128, 768])
    grub0x = di("grub0x", [128, 6], F32)
    grub1h = di("grub1h", [128, 2], F32)
    qkvw1 = di("qkvw1", [18, 128, 768])
    posw2 = di("posw2", [2, 128, 512])
    posT = di("posT", [2, 128, P400])
    qbc = di("qbc", [128, 2], F32)
    kbc = di("kbc", [128, 2], F32)
    drow = di("drow", [1, P400])
    svp4 = di("svp4", [NB, P400], F32)
    ucr = di("ucr", [2, 1, 128])
    c0c = di("c0c", [128, 2], F32)
    hypw = di("hypw", [4, 2, 128, 256])
    hypb = di("hypb", [128, 8], F32)               # col = n*2+mc
    selw = di("selw", [2, 128, 4])
    selb4 = di("selb4", [NB, 4], F32)
    dw1 = di("dw1", [18, 128, 256])
    db1c = di("db1c", [128, 2], F32)
    dw2 = di("dw2", [2, 128, 10])
    db2r = di("db2r", [10, 1], F32)
    eyeb = di("eyeb", [NB, NB])
    onesr = di("onesr", [1, P400])

    y = nc.dram_tensor("y", [NB, NCLS, P400], F32, kind="ExternalOutput").ap()

    dbg = {}

    def mkdbg(name, shape, dt=F32):
        if name in dbg_names:
            dbg[name] = nc.dram_tensor("dbg_" + name, list(shape), dt,
                                       kind="ExternalOutput").ap()

    mkdbg("featsT", [128, 2, NIMG])
    mkdbg("fullT", [128, 18, NB])
    mkdbg("score", [NB, P400])
    mkdbg("oscal", [NB, P400])
    mkdbg("wts", [NB, 4])
    mkdbg("xcmT", [128, 2, NB])
    mkdbg("chosenT", [128, 2, NB, P400])
    mkdbg("act1", [128, 2, PAD])
    mkdbg("act2", [128, 2 * PX])

    with tile.TileContext(nc) as tc, contextlib.ExitStack() as octx:
        main = octx.enter_context(tc.tile_pool(name="main", bufs=1))

        # ---- persistent small tiles ----
        zeros = main.tile([128, 1156], F32, name="zeros")
        nc.vector.memset(zeros[:], 0.0)
        epsb = main.tile([128, 1], F32, name="epsb")
        nc.vector.memset(epsb[:], 1e-3)

        def ld(name, ap, shape, dt=F32R, pool=None):
            t = (pool or main).tile(list(shape), dt, name=name)
            nc.sync.dma_start(t[:], ap[:] if ap.shape == tuple(shape) else ap)
            return t

        w1t = main.tile([K1, 256], F32R, name="w1t")
        nc.sync.dma_start(w1t[:], w1[:])
        w2t = main.tile([128, 18, 256], BF16, name="w2t")
        b1t = ld("b1t", b1c, [128, 2], F32)
        b2t = ld("b2t", b2c, [128, 2], F32)
        ggt = ld("ggt", ggc, [128, 2], F32)
        lnbt = ld("lnbt", lnbc, [128, 2], F32)

        featsT = main.tile([128, 2, NIMG], F32R, name="featsT")
        fullT = main.tile([128, 18, NB], F32R, name="fullT")
        posw2t = main.tile([128, 2, 512], F32R, name="posw2t")
        posTt = main.tile([128, 2, P400], F32R, name="posTt")
        for cc in range(2):
            nc.sync.dma_start(posw2t[:, cc, :], posw2[cc])
            nc.sync.dma_start(posTt[:, cc, :], posT[cc])

        # encoder-phase pools
        ectx = contextlib.ExitStack()
        ep = ectx.enter_context(tc.tile_pool(name="ep", bufs=2))
        xp_ = ectx.enter_context(tc.tile_pool(name="xp", bufs=3))
        xp_ = ectx.enter_context(tc.tile_pool(name="xp", bufs=3))
        ep1 = ectx.enter_context(tc.tile_pool(name="ep1", bufs=2))
        c2pp = ectx.enter_context(tc.tile_pool(name="c2p", bufs=4, space="PSUM"))
        stpp = ectx.enter_context(tc.tile_pool(name="stp", bufs=1, space="PSUM"))
        edram = ectx.enter_context(tc.tile_pool(name="edram", bufs=2, space="DRAM"))
        inv256 = main.tile([1, 1], F32, name="inv256")
        nc.vector.memset(inv256[:], 1.0 / 256)
        ones1f = main.tile([128, 1], F32, name="ones1f")
        nc.vector.memset(ones1f[:], 1.0)
        onesk = main.tile([128, 1], F32R, name="onesk")
        nc.vector.tensor_copy(onesk[:], ones1f[:])

        # padded act1 buffer: [parity][cc] planes, zero borders once
        act1 = main.tile([128, 2, 2, PAD], BF16, name="act1")
        for i in range(4):
            nc.vector.tensor_copy(
                act1[:, i // 2, i % 2, :], zeros[:, 0:PAD])

        def enc_conv(img):
            xt = xp_.tile([K1, PX], F32R, name="xt", tag="xt")
            nc.sync.dma_start(xt[:], xim[img])
            par = img % 2
            # conv1: K=99, four [128,512] psum tiles (shared tag with conv2)
            c1ps = [c2pp.tile([128, 512], F32, name="c1ps", tag="c2ps")
                    for _ in range(4)]
            for co in range(2):
                for q in range(2):
                    nc.tensor.matmul(c1ps[co * 2 + q][:],
                                     w1t[:, co * 128:(co + 1) * 128],
                                     xt[:, q * 512:(q + 1) * 512],
                                     start=True, stop=True)
            for co in range(2):
                for q in range(2):
                    dst = act1[:, par, co, :].rearrange(
                        "p (a b) -> p a b", a=34, b=34)[:, 1 + q * 16: 17 + q * 16, 1:33]
                    nc.scalar.activation(
                        dst, c1ps[co * 2 + q][:].rearrange(
                            "p (a b) -> p a b", a=16, b=32),
                        AF.Relu, bias=b1t[:, co:co + 1])
            if img == 0:
                for k in range(18):
                    nc.sync.dma_start(w2t[:, k, :], w2[k])
            if "act1" in dbg and img == 0:
                for co in range(2):
                    nc.sync.dma_start(dbg["act1"][:, co, :],
                                      act1[:, par, co, :].bitcast(F32))
            # conv2: 18 K-chunks x 2 cout x 2 px-tiles, window APs
            comb = ep.tile([128, 4096], F32R, name="comb", tag="comb")
            for co in range(2):
                for q in range(2):
                    c2ps = c2pp.tile([128, 512], F32, name="c2ps", tag="c2ps")
                    for k in range(18):
                        tap, ci = k // 2, k % 2
                        dy, dx = tap // 3, tap % 3
                        win = act1[:, par, ci, :].rearrange(
                            "p (a b) -> p a b", a=34, b=34)[
                            :, dy + q * 16: dy + q * 16 + 16, dx:dx + 32]
                        nc.tensor.matmul(
                            c2ps[:].rearrange("p (a b) -> p a b", a=16, b=32),
                            w2t[:, k, co * 128:(co + 1) * 128], win,
                            start=(k == 0), stop=(k == 17))
                    nc.scalar.activation(
                        comb[:, co * 1024 + q * 512: co * 1024 + (q + 1) * 512],
                        c2ps[:], AF.Relu, bias=b2t[:, co:co + 1])
            if "act2" in dbg and img == 0:
                nc.sync.dma_start(dbg["act2"][:], comb[:, 0:2048].bitcast(F32))
            # squares on ACT
            nc.scalar.activation(comb[:, 2048:4096], comb[:, 0:2048], AF.Square)
            return comb

        def enc_stats(img, comb):
            # PE ones-matmuls: sum over channels -> psum rows [1,1024] each
            sxp = stpp.tile([1, PX], F32, name="sxp", tag="sxp")
            sxxp = stpp.tile([1, PX], F32, name="sxxp", tag="sxxp")
            for h in range(2):
                for cc in range(2):
                    nc.tensor.matmul(
                        sxp[:, h * 512:(h + 1) * 512], onesk[:],
                        comb[:, cc * 1024 + h * 512: cc * 1024 + (h + 1) * 512],
                        start=(cc == 0), stop=(cc == 1))
            for h in range(2):
                for cc in range(2):
                    nc.tensor.matmul(
                        sxxp[:, h * 512:(h + 1) * 512], onesk[:],
                        comb[:, 2048 + cc * 1024 + h * 512:
                             2048 + cc * 1024 + (h + 1) * 512],
                        start=(cc == 0), stop=(cc == 1))
            return sxp, sxxp

        def enc_final(img, comb, sxp, sxxp):
            # rows: A=mu, B=ex2/var/s, C=sqrt scratch
            rows = ep1.tile([1, 3, PX], F32, name="rows", tag="rows", bufs=1)
            A, Bv, C = rows[:, 0, :], rows[:, 1, :], rows[:, 2, :]
            nc.vector.tensor_scalar_mul(A, sxp[:], inv256[0:1, :])
            nc.vector.tensor_scalar_mul(Bv, sxxp[:], inv256[0:1, :])
            # var = ex2 - mu^2
            nc.vector.tensor_mul(C, A, A)
            nc.vector.tensor_sub(Bv, Bv, C)
            nc.scalar.activation(C, Bv, AF.Abs_reciprocal_sqrt,
                                 bias=epsb[0:1, 0:1])
            Bv, C = C, Bv                                    # s now in C-slot
            nacc = ep1.tile([1, 1], F32, name="nacc", tag="nacc")
            nc.vector.scalar_tensor_tensor(C, A, -1.0, Bv,
                                           op0=ALU.mult, op1=ALU.mult,
                                           accum_out=nacc[:])
            # bounce s and nacc through DRAM for partition broadcast
            sdr = edram.tile([1, PX], F32, name="sdr", tag="sdr")
            ndr = edram.tile([1, 1], F32, name="ndr", tag="ndr")
            nc.sync.dma_start(sdr[:], Bv)
            nc.sync.dma_start(ndr[:], nacc[:])
            sbc = ep1.tile([128, PX], F32, name="sbc", tag="sbc")
            nc.sync.dma_start(sbc[:], sdr[:].to_broadcast([128, PX]))
            nbc = ep1.tile([128, 1], F32, name="nbc", tag="nbc")
            nc.sync.dma_start(nbc[:], ndr[:].to_broadcast([128, 1]))
            bb2 = ep1.tile([128, 2], F32, name="bb2", tag="bb2")
            nc.vector.scalar_tensor_tensor(bb2[:], ggt[:], nbc[:, 0:1],
                                           lnbt[:], op0=ALU.mult, op1=ALU.add)
            qacc = ep1.tile([128, 2], F32, name="qacc", tag="qacc")
            junk = ep1.tile([128, PX], F32, name="junk", tag="junk")
            for cc in range(2):
                nc.vector.scalar_tensor_tensor(
                    junk[:],
                    comb[:, cc * 1024:(cc + 1) * 1024].bitcast(F32), 1.0, sbc[:],
                    op0=ALU.mult, op1=ALU.mult, accum_out=qacc[:, cc:cc + 1])
            for cc in range(2):
                nc.scalar.activation(featsT[:, cc, img:img + 1],
                                     qacc[:, cc:cc + 1], AF.Identity,
                                     bias=bb2[:, cc:cc + 1],
                                     scale=ggt[:, cc:cc + 1])

        prev = None
        for img in range(NIMG):
            comb = enc_conv(img)
            if prev is not None:
                pimg, pcomb = prev
                sxp, sxxp = enc_stats(pimg, pcomb)
                enc_final(pimg, pcomb, sxp, sxxp)
            prev = (img, comb)
            if img == 0:
                qkvw1t = main.tile([128, 18, 768], F32R, name="qkvw1t")
                grukt = main.tile([128, 2, 768], F32R, name="grukt")
                grurkt = main.tile([128, 2, 768], F32R, name="grurkt")
                gb0t = ld("gb0t", grub0x, [128, 6], F32)
                gb1t = ld("gb1t", grub1h, [128, 2], F32)
            k = img - 1
            if 0 <= k < 18 or NIMG < 24:
                if NIMG >= 24:
                    nc.sync.dma_start(qkvw1t[:, k, :], qkvw1[k])
                elif img == 1:
                    for kk in range(18):
                        nc.sync.dma_start(qkvw1t[:, kk, :], qkvw1[kk])
            if img == 19 or (NIMG < 24 and img == 2):
                for cc in range(2):
                    nc.sync.dma_start(grukt[:, cc, :], gruk[cc])
                    nc.sync.dma_start(grurkt[:, cc, :], grurk[cc])
        pimg, pcomb = prev
        sxp, sxxp = enc_stats(pimg, pcomb)
        enc_final(pimg, pcomb, sxp, sxxp)
        if "featsT" in dbg:
            nc.sync.dma_start(dbg["featsT"][:], featsT[:].bitcast(F32))
        ectx.close()
        tailp = octx.enter_context(tc.tile_pool(name="tailp", bufs=1))
        # preload all choice/decoder weights during GRU+attention
        ucrt = tailp.tile([1, 2, 128], F32R, name="ucrt")
        for cc in range(2):
            nc.sync.dma_start(ucrt[:, cc, :], ucr[cc])
        c0t = ld("c0t", c0c, [128, 2], F32, pool=tailp)
        hypwt = tailp.tile([128, 4, 2, 256], F32R, name="hypwt")
        for n in range(4):
            for cc in range(2):
                nc.sync.dma_start(hypwt[:, n, cc, :], hypw[n, cc])
        hypbt = ld("hypbt", hypb, [128, 8], F32, pool=tailp)
        selwt = tailp.tile([128, 2, 4], F32R, name="selwt")
        for cc in range(2):
            nc.sync.dma_start(selwt[:, cc, :], selw[cc])
        selbt = ld("selbt", selb4, [NB, 4], F32, pool=tailp)
        dw1t = tailp.tile([128, 18, 256], F32R, name="dw1t")
        for k in range(18):
            nc.sync.dma_start(dw1t[:, k, :], dw1[k])
        db1t = ld("db1t", db1c, [128, 2], F32, pool=tailp)
        dw2t = tailp.tile([128, 2, 10], F32R, name="dw2t")
        for cc in range(2):
            nc.sync.dma_start(dw2t[:, cc, :], dw2[cc])
        db2t = ld("db2t", db2r, [10, 1], F32, pool=tailp)
        decpad = tailp.tile([128, 2, NB, DPAD], F32R, name="decpad")
        for cc in range(2):
            for b in range(NB):
                nc.vector.tensor_copy(decpad[:, cc, b, :], zeros[:, 0:DPAD])

        # qp/kp projections of pos (independent of GRU; fills feats-wait gap)
        qkctx = contextlib.ExitStack()
        qkpp0 = qkctx.enter_context(tc.tile_pool(name="qkp0", bufs=2, space="PSUM"))
        qpT = tailp.tile([128, 2, P400], F32R, name="qpT")
        kpT = tailp.tile([128, 2, P400], F32R, name="kpT")
        for j, dstT in ((0, qpT), (1, kpT)):
            for mc in range(2):
                ps = qkpp0.tile([128, P400], F32, name="qkps", tag="qkps")
                for cc in range(2):
                    nc.tensor.matmul(ps[:], posw2t[:, cc, j * 256 + mc * 128:
                                                   j * 256 + (mc + 1) * 128],
                                     posTt[:, cc, :],
                                     start=(cc == 0), stop=(cc == 1))
                nc.scalar.activation(dstT[:, mc, :], ps[:], AF.Copy)
        qkctx.close()

        # ================= GRU phase =================
        fqctx = contextlib.ExitStack()
        fqpp = fqctx.enter_context(tc.tile_pool(name="fqp", bufs=1, space="PSUM"))
        fqp = fqpp.tile([NB, 768], F32, name="fqp")

        def fq_mm(k, start, stop):
            for h in range(2):
                nc.tensor.matmul(fqp[:, h * 512:(h + 1) * 512 - (256 if h else 0)],
                                 fullT[:, 16 + k if k < 2 else k, :],
                                 qkvw1t[:, k, h * 512:(h + 1) * 512 - (256 if h else 0)],
                                 start=start, stop=stop)

        actx = contextlib.ExitStack()
        ap_ = actx.enter_context(tc.tile_pool(name="ap", bufs=2))
        mxpp = actx.enter_context(tc.tile_pool(name="mxp", bufs=1, space="PSUM"))
        mhpp = actx.enter_context(tc.tile_pool(name="mhp", bufs=2, space="PSUM"))

        mxp = mxpp.tile([128, 6, NIMG], F32, name="mxp")
        H2 = NIMG // 2
        for hh in range(2):
            for mc in range(6):
                for cc in range(2):
                    nc.tensor.matmul(mxp[:, mc, hh * H2:(hh + 1) * H2],
                                     grukt[:, cc, mc * 128:(mc + 1) * 128],
                                     featsT[:, cc, hh * H2:(hh + 1) * H2],
                                     start=(cc == 0), stop=(cc == 1))
        mxT = main.tile([128, 6, NIMG], F32, name="mxT")
        for mc in range(6):
            nc.scalar.activation(mxT[:, mc, :], mxp[:, mc, :], AF.Identity,
                                 bias=gb0t[:, mc:mc + 1])

        hz = main.tile([128, 2, NB], F32R, name="hz")
        nc.vector.tensor_copy(hz[:], zeros[:, 0:2 * NB].rearrange(
            "p (a b) -> p a b", a=2, b=NB))

        mxTv = mxT[:].rearrange("p m (t b) -> p m t b", b=NB, t=T)
        for t in range(T):
            hprev = hz[:] if t == 0 else fullT[:, 2 * t: 2 * t + 2, :]
            mhp = mhpp.tile([128, 6, NB], F32, name="mhp", tag="mhp")
            for mc in range(6):
                for cc in range(2):
                    nc.tensor.matmul(mhp[:, mc, :],
                                     grurkt[:, cc, mc * 128:(mc + 1) * 128],
                                     hprev[:, cc, :],
                                     start=(cc == 0), stop=(cc == 1))
            if t > 0:
                fq_mm(2 * t, start=(t == 1), stop=False)
                fq_mm(2 * t + 1, start=False, stop=False)
            gt = ap_.tile([128, 6, NB], F32, name="gt", tag="gt")
            # z,r = sigmoid(mx + mh)
            nc.vector.tensor_add(gt[:, 0:4, :], mxTv[:, 0:4, t, :],
                                 mhp[:, 0:4, :])
            nc.scalar.activation(gt[:, 0:4, :], gt[:, 0:4, :], AF.Sigmoid)
            # rh = r*(mh_h + b1h) ; cand = tanh(mx_h + rh)
            for i in range(2):
                nc.vector.scalar_tensor_tensor(
                    gt[:, 4 + i, :], mhp[:, 4 + i, :], gb1t[:, i:i + 1],
                    gt[:, 2 + i, :], op0=ALU.add, op1=ALU.mult)
            nc.vector.tensor_add(gt[:, 4:6, :], gt[:, 4:6, :], mxTv[:, 4:6, t, :])
            nc.scalar.activation(gt[:, 4:6, :], gt[:, 4:6, :], AF.Tanh)
            # h' = cand + z*(h - cand)
            hsub = ap_.tile([128, 2, NB], F32, name="hsub", tag="hsub")
            nc.vector.tensor_sub(hsub[:], hprev.bitcast(F32), gt[:, 4:6, :])
            nc.vector.tensor_mul(hsub[:], gt[:, 0:2, :], hsub[:])
            nc.vector.tensor_add(fullT[:, 2 + 2 * t: 4 + 2 * t, :],
                                 gt[:, 4:6, :], hsub[:])
        fq_mm(16, start=False, stop=False)
        fq_mm(17, start=False, stop=False)
        fq_mm(0, start=False, stop=False)   # state slot = out_7 chunk 0
        fq_mm(1, start=False, stop=True)
        nc.vector.tensor_copy(fullT[:, 0:2, :], fullT[:, 16:18, :])
        if "fullT" in dbg:
            nc.sync.dma_start(dbg["fullT"][:], fullT[:].bitcast(F32))
        fqs = tailp.tile([NB, 768], F32R, name="fqs")
        nc.vector.tensor_copy(fqs[:], fqp[:])
        svf = tailp.tile([NB, 1], F32, name="svf")
        nc.vector.reduce_sum(svf[:], fqp[:, 512:768], axis=AX.X)
        actx.close()
        fqctx.close()

        # ================= attention phase =================
        btx = contextlib.ExitStack()
        bp = btx.enter_context(tc.tile_pool(name="bp", bufs=1))
        trpp = btx.enter_context(tc.tile_pool(name="trp", bufs=2, space="PSUM"))
        scpp = btx.enter_context(tc.tile_pool(name="scp", bufs=1, space="PSUM"))

        qbt = ld("qbt", qbc, [128, 2], F32, pool=bp)
        kbt = ld("kbt", kbc, [128, 2], F32, pool=bp)
        eyet = ld("eyet", eyeb, [NB, NB], F32R, pool=bp)
        onest = ld("onest", onesr, [1, P400], F32R, pool=bp)
        drowt = ld("drowt", drow, [1, P400], F32R, pool=bp)
        svpt = ld("svpt", svp4, [NB, P400], F32, pool=bp)



        # transpose qf,kf chunks to c-layout with bias
        qfT = bp.tile([128, 2, NB], F32R, name="qfT")
        kfT = bp.tile([128, 2, NB], F32R, name="kfT")
        for j, dstT, bt in ((0, qfT, qbt), (1, kfT, kbt)):
            for mc in range(2):
                pst = trpp.tile([128, NB], F32R, name="pst", tag="pst")
                nc.tensor.transpose(
                    pst[:], fqs[:, j * 256 + mc * 128: j * 256 + (mc + 1) * 128],
                    eyet[:])
                nc.scalar.activation(dstT[:, mc, :], pst[:].bitcast(F32),
                                     AF.Identity, bias=bt[:, mc:mc + 1])

        # A[b] = sum_c qfT*kfT via ones-matmul partition reduction
        prod = bp.tile([128, 2 * NB], F32R, name="prod")
        nc.vector.tensor_mul(prod[:].rearrange("p (a b) -> p a b", a=2, b=NB),
                             qfT[:].bitcast(F32), kfT[:].bitcast(F32))
        ared = scpp.tile([1, 2 * NB], F32, name="ared", tag="ared")
        nc.tensor.matmul(ared[:], onesk[:], prod[:], start=True, stop=True)
        areds = bp.tile([1, 2 * NB], F32, name="areds")
        nc.vector.tensor_copy(areds[:], ared[:])
        arow = bp.tile([1, NB], F32R, name="arow")
        nc.vector.tensor_add(arow[:], areds[0:1, 0:NB], areds[0:1, NB:2 * NB])

        # score = B + C + D + A
        scp = scpp.tile([NB, P400], F32, name="scp")
        nc.tensor.matmul(scp[:], qfT[:, 0, :], kpT[:, 0, :], start=True, stop=False)
        nc.tensor.matmul(scp[:], qfT[:, 1, :], kpT[:, 1, :], start=False, stop=False)
        nc.tensor.matmul(scp[:], kfT[:, 0, :], qpT[:, 0, :], start=False, stop=False)
        nc.tensor.matmul(scp[:], kfT[:, 1, :], qpT[:, 1, :], start=False, stop=False)
        nc.tensor.matmul(scp[:], onest[:, 0:NB], drowt[:], start=False, stop=False)
        nc.tensor.matmul(scp[:], arow[:], onest[:], start=False, stop=True)
        if "score" in dbg:
            sc_sb = bp.tile([NB, P400], F32, name="sc_sb")
            nc.vector.tensor_copy(sc_sb[:], scp[:])
            nc.sync.dma_start(dbg["score"][:], sc_sb[:])

        # softmax + o_scal = attn * (svf + svp)
        nmx = bp.tile([NB, 1], F32, name="nmx")
        nc.vector.tensor_reduce(nmx[:], scp[:], axis=AX.X, op=ALU.max, negate=True)
        ex = bp.tile([NB, P400], F32, name="ex")
        se = bp.tile([NB, 1], F32, name="se")
        nc.scalar.activation(ex[:], scp[:], AF.Exp, bias=nmx[:], accum_out=se[:])
        rse = bp.tile([NB, 1], F32, name="rse")
        nc.vector.reciprocal(rse[:], se[:])
        osc = tailp.tile([NB, P400], F32R, name="osc")
        nc.vector.scalar_tensor_tensor(osc[:].bitcast(F32), svpt[:], svf[:],
                                       ex[:], op0=ALU.add, op1=ALU.mult)
        nc.vector.tensor_scalar_mul(osc[:], osc[:].bitcast(F32), rse[:])
        if "oscal" in dbg:
            nc.sync.dma_start(dbg["oscal"][:], osc[:].bitcast(F32))

        btx.close()

        # ============ choice + decoder (merged per-b pipeline) ============
        cctx = contextlib.ExitStack()
        cp = cctx.enter_context(tc.tile_pool(name="cp", bufs=1))
        tnp = cctx.enter_context(tc.tile_pool(name="tnp", bufs=2))
        dwk = cctx.enter_context(tc.tile_pool(name="dwk", bufs=1))
        hppp = cctx.enter_context(tc.tile_pool(name="hpp", bufs=4, space="PSUM"))
        slpp = cctx.enter_context(tc.tile_pool(name="slp", bufs=1, space="PSUM"))
        dpp = cctx.enter_context(tc.tile_pool(name="dpp", bufs=3, space="PSUM"))
        cdram = cctx.enter_context(tc.tile_pool(name="cdram", bufs=1, space="DRAM"))

        # xcT = relu(u (x) o + c0) via K=1 outer-product matmuls
        xcT = cp.tile([128, 2, NB * P400], F32R, name="xcT")
        for b in range(NB):
            oscr = cp.tile([1, P400], F32R, name="oscr", tag="oscr", bufs=2)
            nc.sync.dma_start(oscr[:], osc[b:b + 1, :])
            for cc in range(2):
                xcps = hppp.tile([128, P400], F32, name="xcps", tag="hps")
                nc.tensor.matmul(xcps[:], ucrt[:, cc, :], oscr[:],
                                 start=True, stop=True)
                nc.scalar.activation(
                    xcT[:, cc, b * P400:(b + 1) * P400], xcps[:], AF.Relu,
                    bias=c0t[:, cc:cc + 1])
        # selector weights
        xcmT = cp.tile([128, 2, NB], F32R, name="xcmT")
        with nc.allow_low_precision(reason="f32r is 4-byte; fine"):
            for cc in range(2):
                nc.vector.tensor_reduce(
                    xcmT[:, cc, :],
                    xcT[:, cc, :].rearrange("p (b q) -> p b q", b=NB, q=P400),
                    axis=AX.X, op=ALU.add)
        if "xcmT" in dbg:
            nc.sync.dma_start(dbg["xcmT"][:], xcmT[:].bitcast(F32))
        slp = slpp.tile([NB, 4], F32, name="slp")
        for cc in range(2):
            nc.tensor.matmul(slp[:], xcmT[:, cc, :], selwt[:, cc, :],
                             start=(cc == 0), stop=(cc == 1))
        lg = cp.tile([NB, 4], F32, name="lg")
        nc.vector.tensor_add(lg[:], slp[:], selbt[:])
        nmx2 = cp.tile([NB, 1], F32, name="nmx2")
        nc.vector.tensor_reduce(nmx2[:], lg[:], axis=AX.X, op=ALU.max, negate=True)
        se2 = cp.tile([NB, 1], F32, name="se2")
        nc.scalar.activation(lg[:], lg[:], AF.Exp, bias=nmx2[:], accum_out=se2[:])
        rse2 = cp.tile([NB, 1], F32, name="rse2")
        nc.vector.reciprocal(rse2[:], se2[:])
        nc.vector.tensor_scalar_mul(lg[:], lg[:], rse2[:])
        if "wts" in dbg:
            nc.sync.dma_start(dbg["wts"][:], lg[:])
        wtsd = cdram.tile([NB, 4], F32, name="wtsd")
        nc.sync.dma_start(wtsd[:], lg[:])
        wbc = cp.tile([128, NB * 4], F32, name="wbc")
        for b in range(NB):
            nc.sync.dma_start(wbc[:, b * 4:(b + 1) * 4],
                              wtsd[b:b + 1, :].to_broadcast([128, 4]))

        chosenT = tailp.tile([128, 2, NB, P400], F32R, name="chosenT")
        xcv = xcT[:].rearrange("p c (b q) -> p c b q", b=NB, q=P400)
        for b in range(NB):
            for mc in range(2):
                for n in range(4):
                    hps = hppp.tile([128, P400], F32, name="hps", tag="hps")
                    for cc in range(2):
                        nc.tensor.matmul(hps[:],
                                         hypwt[:, n, cc, mc * 128:(mc + 1) * 128],
                                         xcv[:, cc, b, :],
                                         start=(cc == 0), stop=(cc == 1))
                    tn = tnp.tile([128, P400], F32, name="tn", tag="tn")
                    nc.scalar.activation(tn[:], hps[:], AF.Relu,
                                         bias=hypbt[:, n * 2 + mc: n * 2 + mc + 1])
                    if n == 0:
                        nc.vector.tensor_scalar_mul(
                            chosenT[:, mc, b, :], tn[:],
                            wbc[:, b * 4 + n: b * 4 + n + 1])
                    else:
                        nc.vector.scalar_tensor_tensor(
                            chosenT[:, mc, b, :], tn[:],
                            wbc[:, b * 4 + n: b * 4 + n + 1],
                            chosenT[:, mc, b, :], op0=ALU.mult, op1=ALU.add)
                nc.sync.dma_start(
                    decpad[:, mc, b, :].rearrange(
                        "p (h w) -> p h w", h=22, w=22)[:, 1:21, 1:21],
                    chosenT[:, mc, b, :].rearrange("p (h w) -> p h w", h=20, w=20))
            # decoder for this b
            d1p = [dpp.tile([128, P400], F32, name="d1p", tag="dps")
                   for _ in range(2)]
            korder = list(range(0, 18, 2)) + list(range(1, 18, 2))
            for mc in range(2):
                for i, k in enumerate(korder):
                    tap, ci = k // 2, k % 2
                    dy, dx = tap // 3, tap % 3
                    win = decpad[:, ci, b, :].rearrange(
                        "p (h w) -> p h w", h=22, w=22)[:, dy:dy + 20, dx:dx + 20]
                    nc.tensor.matmul(
                        d1p[mc][:].rearrange("p (a c) -> p a c", a=20, c=20),
                        dw1t[:, k, mc * 128:(mc + 1) * 128], win,
                        start=(i == 0), stop=(i == 17))
            d1t = dwk.tile([128, 2, P400], F32R, name="d1t", tag="d1t")
            for mc in range(2):
                nc.scalar.activation(d1t[:, mc, :], d1p[mc][:], AF.Relu,
                                     bias=db1t[:, mc:mc + 1])
            d2p = dpp.tile([10, P400], F32, name="d2p", tag="dps")
            for cc in range(2):
                nc.tensor.matmul(d2p[:], dw2t[:, cc, :], d1t[:, cc, :],
                                 start=(cc == 0), stop=(cc == 1))
            outb = dwk.tile([10, P400], F32, name="outb", tag="outb")
            nc.scalar.activation(outb[:], d2p[:], AF.Identity, bias=db2t[:])
            nc.sync.dma_start(y[b], outb[:])
        if "chosenT" in dbg:
            nc.sync.dma_start(dbg["chosenT"][:], chosenT[:].bitcast(F32))
        cctx.close()

    nc.compile()
    return nc, dbg


def _prep_shared(inp):
    """Host-side weight reshapes (shared across cores)."""
    f = np.float32
    d = {}
    d["w1"] = np.ascontiguousarray(inp["enc_w1"].reshape(K1, 256), f)
    import ml_dtypes
    d["w2"] = np.ascontiguousarray(inp["enc_w2"].reshape(9, 2, 128, 256)
                                   .reshape(18, 128, 256)).astype(ml_dtypes.bfloat16)
    d["b1c"] = np.ascontiguousarray(inp["enc_b1"].reshape(2, 128).T, f)
    d["b2c"] = np.ascontiguousarray(inp["enc_b2"].reshape(2, 128).T, f)
    d["ggc"] = np.ascontiguousarray((inp["ln_g"] / PX).reshape(2, 128).T, f)
    d["lnbc"] = np.ascontiguousarray(inp["ln_b"].reshape(2, 128).T, f)
    d["gruk"] = np.ascontiguousarray(inp["gru_k"].reshape(2, 128, 768), f)
    d["grurk"] = np.ascontiguousarray(inp["gru_rk"].reshape(2, 128, 768), f)
    b0 = inp["gru_b"][0].reshape(6, 128)
    b1 = inp["gru_b"][1].reshape(6, 128)
    d["grub0x"] = np.ascontiguousarray(
        (b0 + np.vstack([b1[:4], np.zeros((2, 128))])).T, f)
    d["grub1h"] = np.ascontiguousarray(b1[4:6].T, f)
    qkv = np.concatenate([inp["q_w"], inp["k_w"], inp["v_w"]], axis=1)  # [2560,768]
    d["qkvw1"] = np.ascontiguousarray(qkv[:2304].reshape(18, 128, 768), f)
    w2p = qkv[2304:]                                                    # [256,768]
    d["posw2"] = np.ascontiguousarray(
        np.concatenate([w2p[:, 0:256], w2p[:, 256:512]], axis=1)
        .reshape(2, 128, 512), f)
    d["qbc"] = np.ascontiguousarray(inp["q_b"].reshape(2, 128).T, f)
    d["kbc"] = np.ascontiguousarray(inp["k_b"].reshape(2, 128).T, f)
    g = np.linspace(-1.0, 1.0, G, dtype=np.float64)
    yy, xx = np.meshgrid(g, g, indexing="ij")
    posin = np.stack([yy, xx], -1).reshape(P400, 2).astype(f)
    pos = np.tanh(posin @ inp["pos_w"] + inp["pos_b"]).astype(f)        # [400,256]
    d["posT"] = np.ascontiguousarray(pos.T.reshape(2, 128, P400), f)
    qp = pos @ inp["q_w"][2304:]
    kp = pos @ inp["k_w"][2304:]
    vp = pos @ inp["v_w"][2304:]
    d["drow"] = np.ascontiguousarray((qp * kp).sum(1)[None, :], f)
    svp = vp.sum(1) + inp["v_b"].sum()
    d["svp4"] = np.ascontiguousarray(np.tile(svp[None, :], (NB, 1)), f)
    u = inp["o_w"][0] @ inp["ch_in_w"]
    c0 = inp["o_b"] @ inp["ch_in_w"] + inp["ch_in_b"]
    d["ucr"] = np.ascontiguousarray(u.reshape(2, 1, 128), f)
    d["c0c"] = np.ascontiguousarray(c0.reshape(2, 128).T, f)
    d["hypw"] = np.ascontiguousarray(inp["hyp_w"].reshape(4, 2, 128, 256), f)
    d["hypb"] = np.ascontiguousarray(
        inp["hyp_b"].reshape(4, 2, 128).transpose(2, 0, 1).reshape(128, 8), f)
    d["selw"] = np.ascontiguousarray((inp["sel_w"] / P400).reshape(2, 128, 4), f)
    d["selb4"] = np.ascontiguousarray(np.tile(inp["sel_b"][None, :], (NB, 1)), f)
    d["dw1"] = np.ascontiguousarray(inp["dec_w1"].reshape(9, 2, 128, 256)
                                    .reshape(18, 128, 256), f)
    d["db1c"] = np.ascontiguousarray(inp["dec_b1"].reshape(2, 128).T, f)
    d["dw2"] = np.ascontiguousarray(inp["dec_w2"][0, 0].reshape(2, 128, 10), f)
    d["db2r"] = np.ascontiguousarray(inp["dec_b2"].reshape(10, 1), f)
    d["eyeb"] = np.eye(NB, dtype=f)
    d["onesr"] = np.ones((1, P400), f)
    return d


def _prep_core(x_seq, c):
    """Per-core im2col of conv1 input."""
    xs = np.asarray(x_seq[c * NB:(c + 1) * NB], np.float32)  # [NB,T,H,W,C]
    imgs = xs.transpose(1, 0, 2, 3, 4).reshape(NIMG, HI, WI, CIN)  # t-major
    xp = np.pad(imgs, ((0, 0), (1, 1), (1, 1), (0, 0)))
    xim = np.empty((NIMG, K1, PX), np.float32)
    for tap in range(9):
        dy, dx = tap // 3, tap % 3
        win = xp[:, dy:dy + HI, dx:dx + WI, :]               # [NIMG,32,32,11]
        xim[:, tap * CIN:(tap + 1) * CIN, :] = (
            win.transpose(0, 3, 1, 2).reshape(NIMG, CIN, PX))
    return xim


_CACHE = {}


def _get_built(dbg_names=()):
    key = tuple(sorted(dbg_names))
    if key not in _CACHE:
        _CACHE[key] = _build(dbg_names)
    return _CACHE[key]


def run(inputs, trace=False, dbg_names=()):
    nc, dbg = _get_built(dbg_names)
    shared = _prep_shared(inputs)
    in_maps = []
    for c in range(NCORES):
        m = dict(shared)
        m["xim"] = _prep_core(inputs["x_seq"], c)
        in_maps.append(m)
    res = run_bass_kernel_spmd(nc, in_maps, core_ids=list(range(NCORES)),
                               trace=trace)
    out = np.concatenate([res.results[c]["y"] for c in range(NCORES)], axis=0)
    out = np.moveaxis(out.reshape(-1, NCLS, G, G), 1, -1)
    return np.ascontiguousarray(out), res


def kernel(**inputs):
    out, _ = run(inputs)
    return out.astype(np.float32)


# revision 37
# speedup vs baseline: 1.1151x; 1.1151x over previous
"""Trainium2 Bass kernel for nn_Sage14FX (encoder-GRU-attention-choice-decoder).

Sharding: pure data parallel over batch B=32 across 8 NeuronCores (4 b/core).
All heavy matmuls run in fp32r (full PE rate). Host side does layout prep only
(im2col for conv1 input, weight reshapes, tiny weight-only constant folds).
"""
import contextlib
import os
import numpy as np

import concourse.bass as bass
import concourse.bacc as bacc
import concourse.tile as tile
import concourse.mybir as mybir
import concourse.bass_isa as bass_isa
from concourse.bass_utils import run_bass_kernel_spmd

F32 = mybir.dt.float32
F32R = mybir.dt.float32r
AF = mybir.ActivationFunctionType
ALU = mybir.AluOpType
AX = mybir.AxisListType

NCORES = 8
B, T, HI, WI, CIN = 32, 8, 32, 32, 11
HID, G, NCLS = 256, 20, 10
NB = int(os.environ.get("SAGE_NB", "4"))          # batches per core (dev: 1)
NIMG = NB * T
PX = HI * WI                                       # 1024
PAD = 34 * 34                                      # padded 32x32 image
P400 = G * G
DPAD = 22 * 22                                     # padded 20x20 image
K1 = 9 * CIN                                       # 99


def _build(dbg_names=()):
    nc = bacc.Bacc("TRN2", target_bir_lowering=False, debug=False,
                   num_devices=NCORES)

    def di(name, shape, dt=F32R):
        return nc.dram_tensor(name, list(shape), dt, kind="ExternalInput").ap()

    xim = di("xim", [NIMG, K1, PX])
    w1 = di("w1", [K1, 256])
    w2 = di("w2", [18, 128, 256])
    b1c = di("b1c", [128, 2], F32)
    b2c = di("b2c", [128, 2], F32)
    ggc = di("ggc", [128, 2], F32)
    lnbc = di("lnbc", [128, 2], F32)
    gruk = di("gruk", [2, 128, 768])
    grurk = di("grurk", [2, 128, 768])
    grub0x = di("grub0x", [128, 6], F32)
    grub1h = di("grub1h", [128, 2], F32)
    qkvw1 = di("qkvw1", [18, 128, 768])
    posw2 = di("posw2", [2, 128, 512])
    posT = di("posT", [2, 128, P400])
    qbc = di("qbc", [128, 2], F32)
    kbc = di("kbc", [128, 2], F32)
    drow = di("drow", [1, P400])
    svp4 = di("svp4", [NB, P400], F32)
    ucr = di("ucr", [2, 1, 128])
    c0c = di("c0c", [128, 2], F32)
    hypw = di("hypw", [4, 2, 128, 256])
    hypb = di("hypb", [128, 8], F32)               # col = n*2+mc
    selw = di("selw", [2, 128, 4])
    selb4 = di("selb4", [NB, 4], F32)
    dw1 = di("dw1", [18, 128, 256])
    db1c = di("db1c", [128, 2], F32)
    dw2 = di("dw2", [2, 128, 10])
    db2r = di("db2r", [10, 1], F32)
    eyeb = di("eyeb", [NB, NB])
    onesr = di("onesr", [1, P400])

    y = nc.dram_tensor("y", [NB, NCLS, P400], F32, kind="ExternalOutput").ap()

    dbg = {}

    def mkdbg(name, shape, dt=F32):
        if name in dbg_names:
            dbg[name] = nc.dram_tensor("dbg_" + name, list(shape), dt,
                                       kind="ExternalOutput").ap()

    mkdbg("featsT", [128, 2, NIMG])
    mkdbg("fullT", [128, 18, NB])
    mkdbg("score", [NB, P400])
    mkdbg("oscal", [NB, P400])
    mkdbg("wts", [NB, 4])
    mkdbg("xcmT", [128, 2, NB])
    mkdbg("chosenT", [128, 2, NB, P400])
    mkdbg("act1", [128, 2, PAD])
    mkdbg("act2", [128, 2 * PX])

    with tile.TileContext(nc) as tc, contextlib.ExitStack() as octx:
        main = octx.enter_context(tc.tile_pool(name="main", bufs=1))

        # ---- persistent small tiles ----
        zeros = main.tile([128, 1156], F32, name="zeros")
        nc.vector.memset(zeros[:], 0.0)
        epsb = main.tile([128, 1], F32, name="epsb")
        nc.vector.memset(epsb[:], 1e-3)

        def ld(name, ap, shape, dt=F32R, pool=None):
            t = (pool or main).tile(list(shape), dt, name=name)
            nc.sync.dma_start(t[:], ap[:] if ap.shape == tuple(shape) else ap)
            return t

        w1t = main.tile([K1, 256], F32R, name="w1t")
        nc.sync.dma_start(w1t[:], w1[:])
        w2t = main.tile([128, 18, 256], F32R, name="w2t")
        b1t = ld("b1t", b1c, [128, 2], F32)
        b2t = ld("b2t", b2c, [128, 2], F32)
        ggt = ld("ggt", ggc, [128, 2], F32)
        lnbt = ld("lnbt", lnbc, [128, 2], F32)

        featsT = main.tile([128, 2, NIMG], F32R, name="featsT")
        fullT = main.tile([128, 18, NB], F32R, name="fullT")
        posw2t = main.tile([128, 2, 512], F32R, name="posw2t")
        posTt = main.tile([128, 2, P400], F32R, name="posTt")
        for cc in range(2):
            nc.sync.dma_start(posw2t[:, cc, :], posw2[cc])
            nc.sync.dma_start(posTt[:, cc, :], posT[cc])

        # encoder-phase pools
        ectx = contextlib.ExitStack()
        ep = ectx.enter_context(tc.tile_pool(name="ep", bufs=2))
        xp_ = ectx.enter_context(tc.tile_pool(name="xp", bufs=3))
        xp_ = ectx.enter_context(tc.tile_pool(name="xp", bufs=3))
        ep1 = ectx.enter_context(tc.tile_pool(name="ep1", bufs=2))
        c2pp = ectx.enter_context(tc.tile_pool(name="c2p", bufs=4, space="PSUM"))
        stpp = ectx.enter_context(tc.tile_pool(name="stp", bufs=1, space="PSUM"))
        edram = ectx.enter_context(tc.tile_pool(name="edram", bufs=2, space="DRAM"))
        inv256 = main.tile([1, 1], F32, name="inv256")
        nc.vector.memset(inv256[:], 1.0 / 256)
        ones1f = main.tile([128, 1], F32, name="ones1f")
        nc.vector.memset(ones1f[:], 1.0)
        onesk = main.tile([128, 1], F32R, name="onesk")
        nc.vector.tensor_copy(onesk[:], ones1f[:])

        # padded act1 buffer: [parity][cc] planes, zero borders once
        act1 = main.tile([128, 2, 2, PAD], F32R, name="act1")
        for i in range(4):
            nc.vector.tensor_copy(
                act1[:, i // 2, i % 2, :], zeros[:, 0:PAD])

        def enc_conv(img):
            xt = xp_.tile([K1, PX], F32R, name="xt", tag="xt")
            nc.sync.dma_start(xt[:], xim[img])
            par = img % 2
            # conv1: K=99, four [128,512] psum tiles (shared tag with conv2)
            c1ps = [c2pp.tile([128, 512], F32, name="c1ps", tag="c2ps")
                    for _ in range(4)]
            for co in range(2):
                for q in range(2):
                    nc.tensor.matmul(c1ps[co * 2 + q][:],
                                     w1t[:, co * 128:(co + 1) * 128],
                                     xt[:, q * 512:(q + 1) * 512],
                                     start=True, stop=True)
            for co in range(2):
                for q in range(2):
                    dst = act1[:, par, co, :].rearrange(
                        "p (a b) -> p a b", a=34, b=34)[:, 1 + q * 16: 17 + q * 16, 1:33]
                    nc.scalar.activation(
                        dst, c1ps[co * 2 + q][:].rearrange(
                            "p (a b) -> p a b", a=16, b=32),
                        AF.Relu, bias=b1t[:, co:co + 1])
            if img == 0:
                for k in range(18):
                    nc.sync.dma_start(w2t[:, k, :], w2[k])
            if "act1" in dbg and img == 0:
                for co in range(2):
                    nc.sync.dma_start(dbg["act1"][:, co, :],
                                      act1[:, par, co, :].bitcast(F32))
            # conv2: 18 K-chunks x 2 cout x 2 px-tiles, window APs
            comb = ep.tile([128, 4096], F32R, name="comb", tag="comb")
            for co in range(2):
                for q in range(2):
                    c2ps = c2pp.tile([128, 512], F32, name="c2ps", tag="c2ps")
                    for k in range(18):
                        tap, ci = k // 2, k % 2
                        dy, dx = tap // 3, tap % 3
                        win = act1[:, par, ci, :].rearrange(
                            "p (a b) -> p a b", a=34, b=34)[
                            :, dy + q * 16: dy + q * 16 + 16, dx:dx + 32]
                        nc.tensor.matmul(
                            c2ps[:].rearrange("p (a b) -> p a b", a=16, b=32),
                            w2t[:, k, co * 128:(co + 1) * 128], win,
                            start=(k == 0), stop=(k == 17))
                    nc.scalar.activation(
                        comb[:, co * 1024 + q * 512: co * 1024 + (q + 1) * 512],
                        c2ps[:], AF.Relu, bias=b2t[:, co:co + 1])
            if "act2" in dbg and img == 0:
                nc.sync.dma_start(dbg["act2"][:], comb[:, 0:2048].bitcast(F32))
            # squares on ACT
            nc.scalar.activation(comb[:, 2048:4096], comb[:, 0:2048], AF.Square)
            return comb

        def enc_stats(img, comb):
            # PE ones-matmuls: sum over channels -> psum rows [1,1024] each
            sxp = stpp.tile([1, PX], F32, name="sxp", tag="sxp")
            sxxp = stpp.tile([1, PX], F32, name="sxxp", tag="sxxp")
            for h in range(2):
                for cc in range(2):
                    nc.tensor.matmul(
                        sxp[:, h * 512:(h + 1) * 512], onesk[:],
                        comb[:, cc * 1024 + h * 512: cc * 1024 + (h + 1) * 512],
                        start=(cc == 0), stop=(cc == 1))
            for h in range(2):
                for cc in range(2):
                    nc.tensor.matmul(
                        sxxp[:, h * 512:(h + 1) * 512], onesk[:],
                        comb[:, 2048 + cc * 1024 + h * 512:
                             2048 + cc * 1024 + (h + 1) * 512],
                        start=(cc == 0), stop=(cc == 1))
            return sxp, sxxp

        def enc_final(img, comb, sxp, sxxp):
            # rows: A=mu, B=ex2/var/s, C=sqrt scratch
            rows = ep1.tile([1, 3, PX], F32, name="rows", tag="rows", bufs=1)
            A, Bv, C = rows[:, 0, :], rows[:, 1, :], rows[:, 2, :]
            nc.vector.tensor_scalar_mul(A, sxp[:], inv256[0:1, :])
            nc.vector.tensor_scalar_mul(Bv, sxxp[:], inv256[0:1, :])
            # var = ex2 - mu^2
            nc.vector.tensor_mul(C, A, A)
            nc.vector.tensor_sub(Bv, Bv, C)
            nc.scalar.activation(C, Bv, AF.Abs_reciprocal_sqrt,
                                 bias=epsb[0:1, 0:1])
            Bv, C = C, Bv                                    # s now in C-slot
            nacc = ep1.tile([1, 1], F32, name="nacc", tag="nacc")
            nc.vector.scalar_tensor_tensor(C, A, -1.0, Bv,
                                           op0=ALU.mult, op1=ALU.mult,
                                           accum_out=nacc[:])
            # bounce s and nacc through DRAM for partition broadcast
            sdr = edram.tile([1, PX], F32, name="sdr", tag="sdr")
            ndr = edram.tile([1, 1], F32, name="ndr", tag="ndr")
            nc.sync.dma_start(sdr[:], Bv)
            nc.sync.dma_start(ndr[:], nacc[:])
            sbc = ep1.tile([128, PX], F32, name="sbc", tag="sbc")
            nc.sync.dma_start(sbc[:], sdr[:].to_broadcast([128, PX]))
            nbc = ep1.tile([128, 1], F32, name="nbc", tag="nbc")
            nc.sync.dma_start(nbc[:], ndr[:].to_broadcast([128, 1]))
            bb2 = ep1.tile([128, 2], F32, name="bb2", tag="bb2")
            nc.vector.scalar_tensor_tensor(bb2[:], ggt[:], nbc[:, 0:1],
                                           lnbt[:], op0=ALU.mult, op1=ALU.add)
            qacc = ep1.tile([128, 2], F32, name="qacc", tag="qacc")
            junk = ep1.tile([128, PX], F32, name="junk", tag="junk")
            for cc in range(2):
                nc.vector.scalar_tensor_tensor(
                    junk[:],
                    comb[:, cc * 1024:(cc + 1) * 1024].bitcast(F32), 1.0, sbc[:],
                    op0=ALU.mult, op1=ALU.mult, accum_out=qacc[:, cc:cc + 1])
            for cc in range(2):
                nc.scalar.activation(featsT[:, cc, img:img + 1],
                                     qacc[:, cc:cc + 1], AF.Identity,
                                     bias=bb2[:, cc:cc + 1],
                                     scale=ggt[:, cc:cc + 1])

        prev = None
        for img in range(NIMG):
            comb = enc_conv(img)
            if prev is not None:
                pimg, pcomb = prev
                sxp, sxxp = enc_stats(pimg, pcomb)
                enc_final(pimg, pcomb, sxp, sxxp)
            prev = (img, comb)
            if img == 0:
                qkvw1t = main.tile([128, 18, 768], F32R, name="qkvw1t")
                grukt = main.tile([128, 2, 768], F32R, name="grukt")
                grurkt = main.tile([128, 2, 768], F32R, name="grurkt")
                gb0t = ld("gb0t", grub0x, [128, 6], F32)
                gb1t = ld("gb1t", grub1h, [128, 2], F32)
            k = img - 1
            if 0 <= k < 18 or NIMG < 24:
                if NIMG >= 24:
                    nc.sync.dma_start(qkvw1t[:, k, :], qkvw1[k])
                elif img == 1:
                    for kk in range(18):
                        nc.sync.dma_start(qkvw1t[:, kk, :], qkvw1[kk])
            if img == 19 or (NIMG < 24 and img == 2):
                for cc in range(2):
                    nc.sync.dma_start(grukt[:, cc, :], gruk[cc])
                    nc.sync.dma_start(grurkt[:, cc, :], grurk[cc])
        pimg, pcomb = prev
        sxp, sxxp = enc_stats(pimg, pcomb)
        enc_final(pimg, pcomb, sxp, sxxp)
        if "featsT" in dbg:
            nc.sync.dma_start(dbg["featsT"][:], featsT[:].bitcast(F32))
        ectx.close()
        tailp = octx.enter_context(tc.tile_pool(name="tailp", bufs=1))
        # preload all choice/decoder weights during GRU+attention
        ucrt = tailp.tile([1, 2, 128], F32R, name="ucrt")
        for cc in range(2):
            nc.sync.dma_start(ucrt[:, cc, :], ucr[cc])
        c0t = ld("c0t", c0c, [128, 2], F32, pool=tailp)
        hypwt = tailp.tile([128, 4, 2, 256], F32R, name="hypwt")
        for n in range(4):
            for cc in range(2):
                nc.sync.dma_start(hypwt[:, n, cc, :], hypw[n, cc])
        hypbt = ld("hypbt", hypb, [128, 8], F32, pool=tailp)
        selwt = tailp.tile([128, 2, 4], F32R, name="selwt")
        for cc in range(2):
            nc.sync.dma_start(selwt[:, cc, :], selw[cc])
        selbt = ld("selbt", selb4, [NB, 4], F32, pool=tailp)
        dw1t = tailp.tile([128, 18, 256], F32R, name="dw1t")
        for k in range(18):
            nc.sync.dma_start(dw1t[:, k, :], dw1[k])
        db1t = ld("db1t", db1c, [128, 2], F32, pool=tailp)
        dw2t = tailp.tile([128, 2, 10], F32R, name="dw2t")
        for cc in range(2):
            nc.sync.dma_start(dw2t[:, cc, :], dw2[cc])
        db2t = ld("db2t", db2r, [10, 1], F32, pool=tailp)
        decpad = tailp.tile([128, 2, NB, DPAD], F32R, name="decpad")
        for cc in range(2):
            for b in range(NB):
                nc.vector.tensor_copy(decpad[:, cc, b, :], zeros[:, 0:DPAD])

        # qp/kp projections of pos (independent of GRU; fills feats-wait gap)
        qkctx = contextlib.ExitStack()
        qkpp0 = qkctx.enter_context(tc.tile_pool(name="qkp0", bufs=2, space="PSUM"))
        qpT = tailp.tile([128, 2, P400], F32R, name="qpT")
        kpT = tailp.tile([128, 2, P400], F32R, name="kpT")
        for j, dstT in ((0, qpT), (1, kpT)):
            for mc in range(2):
                ps = qkpp0.tile([128, P400], F32, name="qkps", tag="qkps")
                for cc in range(2):
                    nc.tensor.matmul(ps[:], posw2t[:, cc, j * 256 + mc * 128:
                                                   j * 256 + (mc + 1) * 128],
                                     posTt[:, cc, :],
                                     start=(cc == 0), stop=(cc == 1))
                nc.scalar.activation(dstT[:, mc, :], ps[:], AF.Copy)
        qkctx.close()

        # ================= GRU phase =================
        fqctx = contextlib.ExitStack()
        fqpp = fqctx.enter_context(tc.tile_pool(name="fqp", bufs=1, space="PSUM"))
        fqp = fqpp.tile([NB, 768], F32, name="fqp")

        def fq_mm(k, start, stop):
            for h in range(2):
                nc.tensor.matmul(fqp[:, h * 512:(h + 1) * 512 - (256 if h else 0)],
                                 fullT[:, 16 + k if k < 2 else k, :],
                                 qkvw1t[:, k, h * 512:(h + 1) * 512 - (256 if h else 0)],
                                 start=start, stop=stop)

        actx = contextlib.ExitStack()
        ap_ = actx.enter_context(tc.tile_pool(name="ap", bufs=2))
        mxpp = actx.enter_context(tc.tile_pool(name="mxp", bufs=1, space="PSUM"))
        mhpp = actx.enter_context(tc.tile_pool(name="mhp", bufs=2, space="PSUM"))

        mxp = mxpp.tile([128, 6, NIMG], F32, name="mxp")
        H2 = NIMG // 2
        for hh in range(2):
            for mc in range(6):
                for cc in range(2):
                    nc.tensor.matmul(mxp[:, mc, hh * H2:(hh + 1) * H2],
                                     grukt[:, cc, mc * 128:(mc + 1) * 128],
                                     featsT[:, cc, hh * H2:(hh + 1) * H2],
                                     start=(cc == 0), stop=(cc == 1))
        mxT = main.tile([128, 6, NIMG], F32, name="mxT")
        for mc in range(6):
            nc.scalar.activation(mxT[:, mc, :], mxp[:, mc, :], AF.Identity,
                                 bias=gb0t[:, mc:mc + 1])

        hz = main.tile([128, 2, NB], F32R, name="hz")
        nc.vector.tensor_copy(hz[:], zeros[:, 0:2 * NB].rearrange(
            "p (a b) -> p a b", a=2, b=NB))

        mxTv = mxT[:].rearrange("p m (t b) -> p m t b", b=NB, t=T)
        for t in range(T):
            hprev = hz[:] if t == 0 else fullT[:, 2 * t: 2 * t + 2, :]
            mhp = mhpp.tile([128, 6, NB], F32, name="mhp", tag="mhp")
            for mc in range(6):
                for cc in range(2):
                    nc.tensor.matmul(mhp[:, mc, :],
                                     grurkt[:, cc, mc * 128:(mc + 1) * 128],
                                     hprev[:, cc, :],
                                     start=(cc == 0), stop=(cc == 1))
            if t > 0:
                fq_mm(2 * t, start=(t == 1), stop=False)
                fq_mm(2 * t + 1, start=False, stop=False)
            gt = ap_.tile([128, 6, NB], F32, name="gt", tag="gt")
            # z,r = sigmoid(mx + mh)
            nc.vector.tensor_add(gt[:, 0:4, :], mxTv[:, 0:4, t, :],
                                 mhp[:, 0:4, :])
            nc.scalar.activation(gt[:, 0:4, :], gt[:, 0:4, :], AF.Sigmoid)
            # rh = r*(mh_h + b1h) ; cand = tanh(mx_h + rh)
            for i in range(2):
                nc.vector.scalar_tensor_tensor(
                    gt[:, 4 + i, :], mhp[:, 4 + i, :], gb1t[:, i:i + 1],
                    gt[:, 2 + i, :], op0=ALU.add, op1=ALU.mult)
            nc.vector.tensor_add(gt[:, 4:6, :], gt[:, 4:6, :], mxTv[:, 4:6, t, :])
            nc.scalar.activation(gt[:, 4:6, :], gt[:, 4:6, :], AF.Tanh)
            # h' = cand + z*(h - cand)
            hsub = ap_.tile([128, 2, NB], F32, name="hsub", tag="hsub")
            nc.vector.tensor_sub(hsub[:], hprev.bitcast(F32), gt[:, 4:6, :])
            nc.vector.tensor_mul(hsub[:], gt[:, 0:2, :], hsub[:])
            nc.vector.tensor_add(fullT[:, 2 + 2 * t: 4 + 2 * t, :],
                                 gt[:, 4:6, :], hsub[:])
        fq_mm(16, start=False, stop=False)
        fq_mm(17, start=False, stop=False)
        fq_mm(0, start=False, stop=False)   # state slot = out_7 chunk 0
        fq_mm(1, start=False, stop=True)
        nc.vector.tensor_copy(fullT[:, 0:2, :], fullT[:, 16:18, :])
        if "fullT" in dbg:
            nc.sync.dma_start(dbg["fullT"][:], fullT[:].bitcast(F32))
        fqs = tailp.tile([NB, 768], F32R, name="fqs")
        nc.vector.tensor_copy(fqs[:], fqp[:])
        svf = tailp.tile([NB, 1], F32, name="svf")
        nc.vector.reduce_sum(svf[:], fqp[:, 512:768], axis=AX.X)
        actx.close()
        fqctx.close()

        # ================= attention phase =================
        btx = contextlib.ExitStack()
        bp = btx.enter_context(tc.tile_pool(name="bp", bufs=1))
        trpp = btx.enter_context(tc.tile_pool(name="trp", bufs=2, space="PSUM"))
        scpp = btx.enter_context(tc.tile_pool(name="scp", bufs=1, space="PSUM"))

        qbt = ld("qbt", qbc, [128, 2], F32, pool=bp)
        kbt = ld("kbt", kbc, [128, 2], F32, pool=bp)
        eyet = ld("eyet", eyeb, [NB, NB], F32R, pool=bp)
        onest = ld("onest", onesr, [1, P400], F32R, pool=bp)
        drowt = ld("drowt", drow, [1, P400], F32R, pool=bp)
        svpt = ld("svpt", svp4, [NB, P400], F32, pool=bp)



        # transpose qf,kf chunks to c-layout with bias
        qfT = bp.tile([128, 2, NB], F32R, name="qfT")
        kfT = bp.tile([128, 2, NB], F32R, name="kfT")
        for j, dstT, bt in ((0, qfT, qbt), (1, kfT, kbt)):
            for mc in range(2):
                pst = trpp.tile([128, NB], F32R, name="pst", tag="pst")
                nc.tensor.transpose(
                    pst[:], fqs[:, j * 256 + mc * 128: j * 256 + (mc + 1) * 128],
                    eyet[:])
                nc.scalar.activation(dstT[:, mc, :], pst[:].bitcast(F32),
                                     AF.Identity, bias=bt[:, mc:mc + 1])

        # A[b] = sum_c qfT*kfT via ones-matmul partition reduction
        prod = bp.tile([128, 2 * NB], F32R, name="prod")
        nc.vector.tensor_mul(prod[:].rearrange("p (a b) -> p a b", a=2, b=NB),
                             qfT[:].bitcast(F32), kfT[:].bitcast(F32))
        ared = scpp.tile([1, 2 * NB], F32, name="ared", tag="ared")
        nc.tensor.matmul(ared[:], onesk[:], prod[:], start=True, stop=True)
        areds = bp.tile([1, 2 * NB], F32, name="areds")
        nc.vector.tensor_copy(areds[:], ared[:])
        arow = bp.tile([1, NB], F32R, name="arow")
        nc.vector.tensor_add(arow[:], areds[0:1, 0:NB], areds[0:1, NB:2 * NB])

        # score = B + C + D + A
        scp = scpp.tile([NB, P400], F32, name="scp")
        nc.tensor.matmul(scp[:], qfT[:, 0, :], kpT[:, 0, :], start=True, stop=False)
        nc.tensor.matmul(scp[:], qfT[:, 1, :], kpT[:, 1, :], start=False, stop=False)
        nc.tensor.matmul(scp[:], kfT[:, 0, :], qpT[:, 0, :], start=False, stop=False)
        nc.tensor.matmul(scp[:], kfT[:, 1, :], qpT[:, 1, :], start=False, stop=False)
        nc.tensor.matmul(scp[:], onest[:, 0:NB], drowt[:], start=False, stop=False)
        nc.tensor.matmul(scp[:], arow[:], onest[:], start=False, stop=True)
        if "score" in dbg:
            sc_sb = bp.tile([NB, P400], F32, name="sc_sb")
            nc.vector.tensor_copy(sc_sb[:], scp[:])
            nc.sync.dma_start(dbg["score"][:], sc_sb[:])

        # softmax + o_scal = attn * (svf + svp)
        nmx = bp.tile([NB, 1], F32, name="nmx")
        nc.vector.tensor_reduce(nmx[:], scp[:], axis=AX.X, op=ALU.max, negate=True)
        ex = bp.tile([NB, P400], F32, name="ex")
        se = bp.tile([NB, 1], F32, name="se")
        nc.scalar.activation(ex[:], scp[:], AF.Exp, bias=nmx[:], accum_out=se[:])
        rse = bp.tile([NB, 1], F32, name="rse")
        nc.vector.reciprocal(rse[:], se[:])
        osc = tailp.tile([NB, P400], F32R, name="osc")
        nc.vector.scalar_tensor_tensor(osc[:].bitcast(F32), svpt[:], svf[:],
                                       ex[:], op0=ALU.add, op1=ALU.mult)
        nc.vector.tensor_scalar_mul(osc[:], osc[:].bitcast(F32), rse[:])
        if "oscal" in dbg:
            nc.sync.dma_start(dbg["oscal"][:], osc[:].bitcast(F32))

        btx.close()

        # ============ choice + decoder (merged per-b pipeline) ============
        cctx = contextlib.ExitStack()
        cp = cctx.enter_context(tc.tile_pool(name="cp", bufs=1))
        tnp = cctx.enter_context(tc.tile_pool(name="tnp", bufs=2))
        dwk = cctx.enter_context(tc.tile_pool(name="dwk", bufs=1))
        hppp = cctx.enter_context(tc.tile_pool(name="hpp", bufs=4, space="PSUM"))
        slpp = cctx.enter_context(tc.tile_pool(name="slp", bufs=1, space="PSUM"))
        dpp = cctx.enter_context(tc.tile_pool(name="dpp", bufs=3, space="PSUM"))
        cdram = cctx.enter_context(tc.tile_pool(name="cdram", bufs=1, space="DRAM"))

        # xcT = relu(u (x) o + c0) via K=1 outer-product matmuls
        xcT = cp.tile([128, 2, NB * P400], F32R, name="xcT")
        for b in range(NB):
            oscr = cp.tile([1, P400], F32R, name="oscr", tag="oscr", bufs=2)
            nc.sync.dma_start(oscr[:], osc[b:b + 1, :])
            for cc in range(2):
                xcps = hppp.tile([128, P400], F32, name="xcps", tag="hps")
                nc.tensor.matmul(xcps[:], ucrt[:, cc, :], oscr[:],
                                 start=True, stop=True)
                nc.scalar.activation(
                    xcT[:, cc, b * P400:(b + 1) * P400], xcps[:], AF.Relu,
                    bias=c0t[:, cc:cc + 1])
        # selector weights
        xcmT = cp.tile([128, 2, NB], F32R, name="xcmT")
        with nc.allow_low_precision(reason="f32r is 4-byte; fine"):
            for cc in range(2):
                nc.vector.tensor_reduce(
                    xcmT[:, cc, :],
                    xcT[:, cc, :].rearrange("p (b q) -> p b q", b=NB, q=P400),
                    axis=AX.X, op=ALU.add)
        if "xcmT" in dbg:
            nc.sync.dma_start(dbg["xcmT"][:], xcmT[:].bitcast(F32))
        slp = slpp.tile([NB, 4], F32, name="slp")
        for cc in range(2):
            nc.tensor.matmul(slp[:], xcmT[:, cc, :], selwt[:, cc, :],
                             start=(cc == 0), stop=(cc == 1))
        lg = cp.tile([NB, 4], F32, name="lg")
        nc.vector.tensor_add(lg[:], slp[:], selbt[:])
        nmx2 = cp.tile([NB, 1], F32, name="nmx2")
        nc.vector.tensor_reduce(nmx2[:], lg[:], axis=AX.X, op=ALU.max, negate=True)
        se2 = cp.tile([NB, 1], F32, name="se2")
        nc.scalar.activation(lg[:], lg[:], AF.Exp, bias=nmx2[:], accum_out=se2[:])
        rse2 = cp.tile([NB, 1], F32, name="rse2")
        nc.vector.reciprocal(rse2[:], se2[:])
        nc.vector.tensor_scalar_mul(lg[:], lg[:], rse2[:])
        if "wts" in dbg:
            nc.sync.dma_start(dbg["wts"][:], lg[:])
        wtsd = cdram.tile([NB, 4], F32, name="wtsd")
        nc.sync.dma_start(wtsd[:], lg[:])
        wbc = cp.tile([128, NB * 4], F32, name="wbc")
        for b in range(NB):
            nc.sync.dma_start(wbc[:, b * 4:(b + 1) * 4],
                              wtsd[b:b + 1, :].to_broadcast([128, 4]))

        chosenT = tailp.tile([128, 2, NB, P400], F32R, name="chosenT")
        xcv = xcT[:].rearrange("p c (b q) -> p c b q", b=NB, q=P400)
        for b in range(NB):
            for mc in range(2):
                for n in range(4):
                    hps = hppp.tile([128, P400], F32, name="hps", tag="hps")
                    for cc in range(2):
                        nc.tensor.matmul(hps[:],
                                         hypwt[:, n, cc, mc * 128:(mc + 1) * 128],
                                         xcv[:, cc, b, :],
                                         start=(cc == 0), stop=(cc == 1))
                    tn = tnp.tile([128, P400], F32, name="tn", tag="tn")
                    nc.scalar.activation(tn[:], hps[:], AF.Relu,
                                         bias=hypbt[:, n * 2 + mc: n * 2 + mc + 1])
                    if n == 0:
                        nc.vector.tensor_scalar_mul(
                            chosenT[:, mc, b, :], tn[:],
                            wbc[:, b * 4 + n: b * 4 + n + 1])
                    else:
                        nc.vector.scalar_tensor_tensor(
                            chosenT[:, mc, b, :], tn[:],
                            wbc[:, b * 4 + n: b * 4 + n + 1],
                            chosenT[:, mc, b, :], op0=ALU.mult, op1=ALU.add)
                nc.sync.dma_start(
                    decpad[:, mc, b, :].rearrange(
                        "p (h w) -> p h w", h=22, w=22)[:, 1:21, 1:21],
                    chosenT[:, mc, b, :].rearrange("p (h w) -> p h w", h=20, w=20))
            # decoder for this b
            d1p = [dpp.tile([128, P400], F32, name="d1p", tag="dps")
                   for _ in range(2)]
            korder = list(range(0, 18, 2)) + list(range(1, 18, 2))
            for mc in range(2):
                for i, k in enumerate(korder):
                    tap, ci = k // 2, k % 2
                    dy, dx = tap // 3, tap % 3
                    win = decpad[:, ci, b, :].rearrange(
                        "p (h w) -> p h w", h=22, w=22)[:, dy:dy + 20, dx:dx + 20]
                    nc.tensor.matmul(
                        d1p[mc][:].rearrange("p (a c) -> p a c", a=20, c=20),
                        dw1t[:, k, mc * 128:(mc + 1) * 128], win,
                        start=(i == 0), stop=(i == 17))
            d1t = dwk.tile([128, 2, P400], F32R, name="d1t", tag="d1t")
            for mc in range(2):
                nc.scalar.activation(d1t[:, mc, :], d1p[mc][:], AF.Relu,
                                     bias=db1t[:, mc:mc + 1])
            d2p = dpp.tile([10, P400], F32, name="d2p", tag="dps")
            for cc in range(2):
                nc.tensor.matmul(d2p[:], dw2t[:, cc, :], d1t[:, cc, :],
                                 start=(cc == 0), stop=(cc == 1))
            outb = dwk.tile([10, P400], F32, name="outb", tag="outb")
            nc.scalar.activation(outb[:], d2p[:], AF.Identity, bias=db2t[:])
            nc.sync.dma_start(y[b], outb[:])
        if "chosenT" in dbg:
            nc.sync.dma_start(dbg["chosenT"][:], chosenT[:].bitcast(F32))
        cctx.close()

    nc.compile()
    return nc, dbg


def _prep_shared(inp):
    """Host-side weight reshapes (shared across cores)."""
    f = np.float32
    d = {}
    d["w1"] = np.ascontiguousarray(inp["enc_w1"].reshape(K1, 256), f)
    d["w2"] = np.ascontiguousarray(inp["enc_w2"].reshape(9, 2, 128, 256)
                                   .reshape(18, 128, 256), f)
    d["b1c"] = np.ascontiguousarray(inp["enc_b1"].reshape(2, 128).T, f)
    d["b2c"] = np.ascontiguousarray(inp["enc_b2"].reshape(2, 128).T, f)
    d["ggc"] = np.ascontiguousarray((inp["ln_g"] / PX).reshape(2, 128).T, f)
    d["lnbc"] = np.ascontiguousarray(inp["ln_b"].reshape(2, 128).T, f)
    d["gruk"] = np.ascontiguousarray(inp["gru_k"].reshape(2, 128, 768), f)
    d["grurk"] = np.ascontiguousarray(inp["gru_rk"].reshape(2, 128, 768), f)
    b0 = inp["gru_b"][0].reshape(6, 128)
    b1 = inp["gru_b"][1].reshape(6, 128)
    d["grub0x"] = np.ascontiguousarray(
        (b0 + np.vstack([b1[:4], np.zeros((2, 128))])).T, f)
    d["grub1h"] = np.ascontiguousarray(b1[4:6].T, f)
    qkv = np.concatenate([inp["q_w"], inp["k_w"], inp["v_w"]], axis=1)  # [2560,768]
    d["qkvw1"] = np.ascontiguousarray(qkv[:2304].reshape(18, 128, 768), f)
    w2p = qkv[2304:]                                                    # [256,768]
    d["posw2"] = np.ascontiguousarray(
        np.concatenate([w2p[:, 0:256], w2p[:, 256:512]], axis=1)
        .reshape(2, 128, 512), f)
    d["qbc"] = np.ascontiguousarray(inp["q_b"].reshape(2, 128).T, f)
    d["kbc"] = np.ascontiguousarray(inp["k_b"].reshape(2, 128).T, f)
    g = np.linspace(-1.0, 1.0, G, dtype=np.float64)
    yy, xx = np.meshgrid(g, g, indexing="ij")
    posin = np.stack([yy, xx], -1).reshape(P400, 2).astype(f)
    pos = np.tanh(posin @ inp["pos_w"] + inp["pos_b"]).astype(f)        # [400,256]
    d["posT"] = np.ascontiguousarray(pos.T.reshape(2, 128, P400), f)
    qp = pos @ inp["q_w"][2304:]
    kp = pos @ inp["k_w"][2304:]
    vp = pos @ inp["v_w"][2304:]
    d["drow"] = np.ascontiguousarray((qp * kp).sum(1)[None, :], f)
    svp = vp.sum(1) + inp["v_b"].sum()
    d["svp4"] = np.ascontiguousarray(np.tile(svp[None, :], (NB, 1)), f)
    u = inp["o_w"][0] @ inp["ch_in_w"]
    c0 = inp["o_b"] @ inp["ch_in_w"] + inp["ch_in_b"]
    d["ucr"] = np.ascontiguousarray(u.reshape(2, 1, 128), f)
    d["c0c"] = np.ascontiguousarray(c0.reshape(2, 128).T, f)
    d["hypw"] = np.ascontiguousarray(inp["hyp_w"].reshape(4, 2, 128, 256), f)
    d["hypb"] = np.ascontiguousarray(
        inp["hyp_b"].reshape(4, 2, 128).transpose(2, 0, 1).reshape(128, 8), f)
    d["selw"] = np.ascontiguousarray((inp["sel_w"] / P400).reshape(2, 128, 4), f)
    d["selb4"] = np.ascontiguousarray(np.tile(inp["sel_b"][None, :], (NB, 1)), f)
    d["dw1"] = np.ascontiguousarray(inp["dec_w1"].reshape(9, 2, 128, 256)
                                    .reshape(18, 128, 256), f)
    d["db1c"] = np.ascontiguousarray(inp["dec_b1"].reshape(2, 128).T, f)
    d["dw2"] = np.ascontiguousarray(inp["dec_w2"][0, 0].reshape(2, 128, 10), f)
    d["db2r"] = np.ascontiguousarray(inp["dec_b2"].reshape(10, 1), f)
    d["eyeb"] = np.eye(NB, dtype=f)
    d["onesr"] = np.ones((1, P400), f)
    return d


def _prep_core(x_seq, c):
    """Per-core im2col of conv1 input."""
    xs = np.asarray(x_seq[c * NB:(c + 1) * NB], np.float32)  # [NB,T,H,W,C]
    imgs = xs.transpose(1, 0, 2, 3, 4).reshape(NIMG, HI, WI, CIN)  # t-major
    xp = np.pad(imgs, ((0, 0), (1, 1), (1, 1), (0, 0)))
    xim = np.empty((NIMG, K1, PX), np.float32)
    for tap in range(9):
        dy, dx = tap // 3, tap % 3
        win = xp[:, dy:dy + HI, dx:dx + WI, :]               # [NIMG,32,32,11]
        xim[:, tap * CIN:(tap + 1) * CIN, :] = (
            win.transpose(0, 3, 1, 2).reshape(NIMG, CIN, PX))
    return xim


_CACHE = {}


def _get_built(dbg_names=()):
    key = tuple(sorted(dbg_names))
    if key not in _CACHE:
        _CACHE[key] = _build(dbg_names)
    return _CACHE[key]


def run(inputs, trace=False, dbg_names=()):
    nc, dbg = _get_built(dbg_names)
    shared = _prep_shared(inputs)
    in_maps = []
    for c in range(NCORES):
        m = dict(shared)
        m["xim"] = _prep_core(inputs["x_seq"], c)
        in_maps.append(m)
    res = run_bass_kernel_spmd(nc, in_maps, core_ids=list(range(NCORES)),
                               trace=trace)
    out = np.concatenate([res.results[c]["y"] for c in range(NCORES)], axis=0)
    out = np.moveaxis(out.reshape(-1, NCLS, G, G), 1, -1)
    return np.ascontiguousarray(out), res


def kernel(**inputs):
    out, _ = run(inputs)
    return out.astype(np.float32)
